# revision 1
# baseline (speedup 1.0000x reference)
"""Trainium2 Bass kernel for nn_NewModel_66176856097442 (TransE-style loss).

Strategy (data-parallel over the batch of triples):
  - B = 262144 triples sharded as 32768/core across 8 NeuronCores.
  - Embedding tables replicated per core in HBM. predVec+predBias fused on
    host into one fp16 table row [128 vec | bias | 3 pad] = 132 fp16 = 264B,
    so one gathered row brings vector and bias together.
  - relEmb fused the same way: [128 vec | 0 | m_hypo | m_hyper | m_syn]
    so the per-triple relation gather also brings the category masks.
  - Rows gathered from HBM with gpsimd indirect DMA, 128 rows per call
    (one row per partition -- the validated indirect_dma_start shape).
  - Distances/scores computed on-chip (DVE fp16 elementwise + f32 reduce),
    per-core partial sum of cost returned as [128,1]; host sums / B.
"""

import sys

sys.path.insert(0, "/opt/trn_rl_repo")

import numpy as np

import concourse.bass as bass
from concourse import bacc
import concourse.tile as tile
from concourse import mybir
from concourse.bass import IndirectOffsetOnAxis
from concourse.bass_utils import run_bass_kernel_spmd

F32 = mybir.dt.float32
F16 = mybir.dt.float16
I32 = mybir.dt.int32

NUM_ENTITY = 100000
NUM_RELATION = 18
D = 128
DF = 132                   # fused row: 128 vec + bias + 3 pad (fp16)
B = 262144
N_CORES = 8
NB = B // N_CORES          # triples per core
P = 128                    # partitions
NBK = NB // P              # triples per partition per core (256)
MARGIN = 1.0

HYPONYM = (4, 6)
HYPERNYM = (3, 5)
SYNONYM = (0, 1, 13, 17)


def build_bass(nb=NB):
    """Per-core Bass kernel; nb = triples handled by this core."""
    nbk = nb // P

    nc = bacc.Bacc("TRN2", target_bir_lowering=False, debug=True)

    vec_t = nc.declare_dram_parameter("vec", [NUM_ENTITY, DF], F16, isOutput=False)
    rel_t = nc.declare_dram_parameter("relemb", [NUM_RELATION, DF], F16, isOutput=False)
    li_t = nc.declare_dram_parameter("li", [P, nbk], I32, isOutput=False)
    ri_t = nc.declare_dram_parameter("ri", [P, nbk], I32, isOutput=False)
    nli_t = nc.declare_dram_parameter("nli", [P, nbk], I32, isOutput=False)
    nri_t = nc.declare_dram_parameter("nri", [P, nbk], I32, isOutput=False)
    rel_i_t = nc.declare_dram_parameter("reli", [P, nbk], I32, isOutput=False)
    out_t = nc.declare_dram_parameter("psum_out", [P, 1], F32, isOutput=True)

    with tile.TileContext(nc) as tc:
        with (
            tc.tile_pool(name="persist", bufs=1) as persist,
            tc.tile_pool(name="gather", bufs=3) as gpool,
            tc.tile_pool(name="scratch", bufs=2) as spool,
            tc.tile_pool(name="final", bufs=1) as fpool,
        ):
            # ---- load all index arrays to SBUF once ----
            li = persist.tile([P, nbk], I32, name="li")
            ri = persist.tile([P, nbk], I32, name="ri")
            nli = persist.tile([P, nbk], I32, name="nli")
            nri = persist.tile([P, nbk], I32, name="nri")
            reli = persist.tile([P, nbk], I32, name="reli")
            nc.sync.dma_start(out=li[:], in_=li_t[:])
            nc.sync.dma_start(out=ri[:], in_=ri_t[:])
            nc.sync.dma_start(out=nli[:], in_=nli_t[:])
            nc.sync.dma_start(out=nri[:], in_=nri_t[:])
            nc.sync.dma_start(out=reli[:], in_=rel_i_t[:])

            # per-triple accumulators: s1,s2,s3 = ||u_k||^2, t1,t2,t3 with +re
            S = [persist.tile([P, nbk], F32, name=f"S{i}") for i in range(6)]
            # gathered biases and masks, one column per triple
            lbf = persist.tile([P, nbk], F16, name="lbf")
            rbf = persist.tile([P, nbk], F16, name="rbf")
            nlbf = persist.tile([P, nbk], F16, name="nlbf")
            nrbf = persist.tile([P, nbk], F16, name="nrbf")
            m_hypo = persist.tile([P, nbk], F16, name="m_hypo")
            m_hyper = persist.tile([P, nbk], F16, name="m_hyper")
            m_syn = persist.tile([P, nbk], F16, name="m_syn")

            # one gather call per column j: 128 rows land as [P, DF]
            for j in range(nbk):
                jsl = slice(j, j + 1)
                lv = gpool.tile([P, DF], F16, name="lv", tag="lv")
                rv = gpool.tile([P, DF], F16, name="rv", tag="rv")
                nlv = gpool.tile([P, DF], F16, name="nlv", tag="nlv")
                nrv = gpool.tile([P, DF], F16, name="nrv", tag="nrv")
                re = gpool.tile([P, DF], F16, name="re", tag="re")
                for vt, ixt, table in (
                    (lv, li, vec_t),
                    (rv, ri, vec_t),
                    (nlv, nli, vec_t),
                    (nrv, nri, vec_t),
                    (re, reli, rel_t),
                ):
                    nc.gpsimd.indirect_dma_start(
                        out=vt[:],
                        out_offset=None,
                        in_=table[:],
                        in_offset=IndirectOffsetOnAxis(ap=ixt[:, jsl], axis=0),
                    )
                # stash biases / masks for the final phase
                nc.vector.tensor_copy(lbf[:, jsl], lv[:, 128:129])
                nc.vector.tensor_copy(rbf[:, jsl], rv[:, 128:129])
                nc.vector.tensor_copy(nlbf[:, jsl], nlv[:, 128:129])
                nc.vector.tensor_copy(nrbf[:, jsl], nrv[:, 128:129])
                nc.vector.tensor_copy(m_hypo[:, jsl], re[:, 129:130])
                nc.vector.tensor_copy(m_hyper[:, jsl], re[:, 130:131])
                nc.vector.tensor_copy(m_syn[:, jsl], re[:, 131:132])

                for k, (a, b) in enumerate(((lv, rv), (nlv, rv), (lv, nrv))):
                    u = spool.tile([P, D], F16, name="u", tag="u")
                    v = spool.tile([P, D], F16, name="v", tag="v")
                    sq = spool.tile([P, D], F16, name="sq", tag="sq")
                    sq2 = spool.tile([P, D], F16, name="sq2", tag="sq2")
                    nc.vector.tensor_sub(u[:], a[:, 0:D], b[:, 0:D])
                    nc.vector.tensor_add(v[:], u[:], re[:, 0:D])
                    nc.vector.tensor_mul(sq[:], u[:], u[:])
                    nc.vector.tensor_reduce(
                        out=S[k][:, jsl], in_=sq[:], axis=mybir.AxisListType.X,
                        op=mybir.AluOpType.add,
                    )
                    nc.vector.tensor_mul(sq2[:], v[:], v[:])
                    nc.vector.tensor_reduce(
                        out=S[3 + k][:, jsl], in_=sq2[:], axis=mybir.AxisListType.X,
                        op=mybir.AluOpType.add,
                    )

            # ================= final phase on [P, nbk] tiles =================
            f = lambda nm: fpool.tile([P, nbk], F32, name=nm)

            dist = []
            for i in range(6):
                dt_ = f(f"d{i}")
                nc.scalar.sqrt(dt_[:], S[i][:])
                dist.append(dt_)

            b1, b2, b3 = f("b1"), f("b2"), f("b3")
            nc.vector.tensor_sub(b1[:], lbf[:], rbf[:])
            nc.vector.tensor_sub(b2[:], nlbf[:], rbf[:])
            nc.vector.tensor_sub(b3[:], lbf[:], nrbf[:])

            mh, mr, ms = f("mh"), f("mr"), f("ms")
            nc.vector.tensor_copy(mh[:], m_hypo[:])
            nc.vector.tensor_copy(mr[:], m_hyper[:])
            nc.vector.tensor_copy(ms[:], m_syn[:])
            mt = f("mt")  # m_trans = 1 - mh - mr - ms
            nc.vector.tensor_add(mt[:], mh[:], mr[:])
            nc.vector.tensor_add(mt[:], mt[:], ms[:])
            nc.vector.tensor_scalar(
                mt[:], mt[:], -1.0, 1.0,
                op0=mybir.AluOpType.mult, op1=mybir.AluOpType.add,
            )

            scores = []
            for k, bk in enumerate((b1, b2, b3)):
                dk, tk = dist[k], dist[3 + k]
                hyp = f("hyp")
                nc.vector.tensor_sub(hyp[:], dk[:], bk[:])
                nc.vector.tensor_scalar_max(hyp[:], hyp[:], 0.0)
                hyr = f("hyr")
                nc.vector.tensor_add(hyr[:], dk[:], bk[:])
                nc.vector.tensor_scalar_max(hyr[:], hyr[:], 0.0)
                syn = f("syn")
                # |b| = max(b * -1, b)
                nc.vector.scalar_tensor_tensor(
                    syn[:], bk[:], -1.0, bk[:],
                    op0=mybir.AluOpType.mult, op1=mybir.AluOpType.max,
                )
                nc.vector.tensor_add(syn[:], syn[:], dk[:])
                sc = f(f"sc{k}")
                nc.vector.tensor_mul(sc[:], mh[:], hyp[:])
                nc.vector.tensor_mul(hyp[:], mr[:], hyr[:])
                nc.vector.tensor_add(sc[:], sc[:], hyp[:])
                nc.vector.tensor_mul(hyp[:], ms[:], syn[:])
                nc.vector.tensor_add(sc[:], sc[:], hyp[:])
                nc.vector.tensor_mul(hyp[:], mt[:], tk[:])
                nc.vector.tensor_add(sc[:], sc[:], hyp[:])
                scores.append(sc)

            q2, q3 = f("q2"), f("q3")
            nc.vector.tensor_sub(q2[:], scores[0][:], scores[1][:])
            nc.vector.tensor_scalar(
                q2[:], q2[:], MARGIN, 0.0,
                op0=mybir.AluOpType.add, op1=mybir.AluOpType.max,
            )
            nc.vector.tensor_sub(q3[:], scores[0][:], scores[2][:])
            nc.vector.tensor_scalar(
                q3[:], q3[:], MARGIN, 0.0,
                op0=mybir.AluOpType.add, op1=mybir.AluOpType.max,
            )
            nc.vector.tensor_add(q2[:], q2[:], q3[:])
            part = fpool.tile([P, 1], F32, name="part")
            nc.vector.tensor_reduce(
                out=part[:], in_=q2[:], axis=mybir.AxisListType.X,
                op=mybir.AluOpType.add,
            )
            nc.sync.dma_start(out=out_t[:], in_=part[:])

    nc.finalize()
    return nc


_NC_CACHE = {}


def _get_nc(nb=NB):
    if nb not in _NC_CACHE:
        _NC_CACHE[nb] = build_bass(nb)
    return _NC_CACHE[nb]


def _fused_tables(inputs):
    vec = np.asarray(inputs["predVec"], dtype=np.float32)
    biasv = np.asarray(inputs["predBias"], dtype=np.float32).reshape(NUM_ENTITY)
    relemb = np.asarray(inputs["relEmb"], dtype=np.float32)

    fused = np.zeros((NUM_ENTITY, DF), dtype=np.float16)
    fused[:, 0:D] = vec.astype(np.float16)
    fused[:, D] = biasv.astype(np.float16)

    relf = np.zeros((NUM_RELATION, DF), dtype=np.float16)
    relf[:, 0:D] = relemb.astype(np.float16)
    rids = np.arange(NUM_RELATION)
    relf[:, 129] = np.isin(rids, HYPONYM).astype(np.float16)
    relf[:, 130] = np.isin(rids, HYPERNYM).astype(np.float16)
    relf[:, 131] = np.isin(rids, SYNONYM).astype(np.float16)
    return fused, relf


def _prep_inputs(inputs, nb=NB, n_cores=N_CORES):
    fused, relf = _fused_tables(inputs)

    def shard(name):
        arr = np.asarray(inputs[name], dtype=np.int32)
        return [
            np.ascontiguousarray(arr[c * nb:(c + 1) * nb].reshape(P, nb // P))
            for c in range(n_cores)
        ]

    li = shard("leftEnIndices")
    ri = shard("rightEnIndices")
    nli = shard("negLeftEnIndices")
    nri = shard("negRightEnIndices")
    reli = shard("relIndices")

    return [
        {
            "vec": fused, "relemb": relf,
            "li": li[c], "ri": ri[c], "nli": nli[c], "nri": nri[c],
            "reli": reli[c],
        }
        for c in range(n_cores)
    ]


def run(inputs, trace=False):
    nc = _get_nc(NB)
    in_maps = _prep_inputs(inputs)
    res = run_bass_kernel_spmd(nc, in_maps, core_ids=list(range(N_CORES)), trace=trace)
    total = sum(float(r["psum_out"].astype(np.float64).sum()) for r in res.results)
    out = np.float32(total / B)
    return np.asarray(out, dtype=np.float32), res


def kernel(**inputs) -> np.ndarray:
    out, _ = run(inputs, trace=False)
    return out

